# revision 24
# baseline (speedup 1.0000x reference)
"""Slot-attention kernel for Trainium2, SPMD over 8 NeuronCores (v10).

Reference computation (per batch element b):
  query[b,n,:] = q[n,b,:] @ qw[n]          (n = 32 query slots)
  keyp [b,m,:] = k[m,b,:] @ kw[m]          (m = 32 key slots)
  value[b,m,:] = k[m,b,:] @ vw[m]
  logits[b,n,m] = query[b,n,:]·keyp[b,m,:] / 16
  attn = softmax_m(logits)
  out[n,b,:] = sum_m attn[b,n,m] * value[b,m,:]

Sharding: data-parallel over batch (4096 -> 512 per core), weights replicated.

Pipeline layout (per core, two batch halves of 256):
  - dedicated DMA paths: SP HWDGE ring = activations only (so the next
    half's inputs prefetch during phase B); ACT HWDGE ring = weights +
    output stores; gpsimd SWDGE = the V32Q scatter only.
  - q-projection weights stay SBUF-resident (loaded once, chunk-
    interleaved into the first sg iterations); k/v weights stream per sg.
  - phase A: per 2-slot unit, V projections first (psum [b,o]) so the
    V32Q scatter drains early, then Q/K projections (psum [a,b]);
    evacuations are merged [128,1024] ops alternating ScalarE/VectorE;
    the 1/16 temperature is split as 1/4 on Q and 1/4 on K.
  - V32Q[32j+m, g, o] (value for batch 64j+g) is ping-pong buffered
    across halves so half 1's scatter never waits on half 0's attn@V.
  - phase B: logits as col-tiled 4-batch waves + exp on ScalarE (softmax
    without max-subtraction; |logits| <= ~2.5), rowsums 8 groups per DVE
    reduce, attn^T via 4-group 32x32 block transposes, attn@V as diagonal
    tile-packed matmuls, psum->sbuf with folded 1/rowsum, bf16 output via
    one 128-partition DMA per 8 groups.
"""

import numpy as np
import ml_dtypes

import concourse.bass as bass
from concourse import bacc
import concourse.mybir as mybir
import concourse.tile as tile
from concourse.bass_utils import run_bass_kernel_spmd

BF16 = mybir.dt.bfloat16
F32 = mybir.dt.float32

NQ = 32          # query slots
NK = 32          # key slots
D = 256          # input dim (contraction of projections)
A = 256          # attn dim (contraction of logits)
O = 256          # out dim
BS = 4096
N_CORES = 8
BS_CORE = BS // N_CORES   # 512
N_HALVES = 2
B_H = BS_CORE // N_HALVES  # 256
N_GROUPS = B_H // 4        # 64 groups of 4 batches per half
N_QUADS = N_GROUPS // 4    # 16


def build_kernel():
    nc = bacc.Bacc()

    xH = nc.declare_dram_parameter(
        "xH", [N_HALVES, 16, 128, 2, 2, 2, B_H], BF16, isOutput=False
    )  # [half, sg, p, qk, s, c, b]
    wqk = nc.declare_dram_parameter(
        "wqk", [128, NQ, 2, 2, A], BF16, isOutput=False
    )  # [p, slot, c, (q k), a]
    wvH = nc.declare_dram_parameter(
        "wvH", [16, 128, 2, 2, O], BF16, isOutput=False
    )  # [sg, p, s, c, o]
    # batch = 256*half + 64*j + g; host un-permutes to [n, b, o]
    out = nc.declare_dram_parameter(
        "out", [N_HALVES, 4, NQ, N_GROUPS, O], BF16, isOutput=True)
    out_r = out.rearrange("h j n g o -> h (j n) g o")

    with tile.TileContext(nc) as tc:
        with (
            tc.tile_pool(name="wpool", bufs=1) as wpool,
            tc.tile_pool(name="xin", bufs=3) as xin,
            tc.tile_pool(name="wvin", bufs=3) as wvin,
            tc.tile_pool(name="big", bufs=1) as big,
            tc.tile_pool(name="v32", bufs=1) as v32,
            tc.tile_pool(name="vn", bufs=4) as vn,
            tc.tile_pool(name="etp", bufs=4) as etp,
            tc.tile_pool(name="smp", bufs=4) as smp,
            tc.tile_pool(name="outp", bufs=2) as outp,
            # one 8-bank psum ring shared by all matmul outputs: deep
            # rotation so projections of sg+1 never wait on sg's drains
            tc.tile_pool(name="mm_ps", bufs=8, space="PSUM") as mm_ps,
        ):
            wq = wpool.tile([128, NQ, 2, 2, A], BF16, tag="wq")

            def load_wq_chunk(ch):
                nc.scalar.dma_start(
                    out=wq[:, 4 * ch:4 * (ch + 1)],
                    in_=wqk[:, 4 * ch:4 * (ch + 1)],
                )

            evac_flip = [0]

            def evac(dst, src, scale=None):
                e = evac_flip[0] = 1 - evac_flip[0]
                if scale is None:
                    if e:
                        nc.scalar.copy(out=dst, in_=src)
                    else:
                        nc.vector.tensor_copy(out=dst, in_=src)
                else:
                    if e:
                        nc.scalar.mul(dst, src, scale)
                    else:
                        nc.vector.tensor_scalar_mul(out=dst, in0=src,
                                                    scalar1=scale)

            state = {}

            def open_proj(half):
                QTs = big.tile([128, 2, NQ, B_H], BF16, tag="QTs")
                KTs = big.tile([128, 2, NK, B_H], BF16, tag="KTs")
                V32Q = v32.tile([128, N_GROUPS, O], BF16, tag="V32Q")
                state[half] = {
                    "QTs": QTs, "KTs": KTs, "V32Q": V32Q,
                    "V32Q_r": V32Q.rearrange("(bc q) g o -> bc q g o", bc=2),
                }

            def open_soft(half):
                E = big.tile([128, N_GROUPS, NK], BF16, tag="E")
                rs = big.tile([128, N_GROUPS], F32, tag="rs")
                state[half]["E"] = E
                state[half]["rs"] = rs

            def phase_a_sg(half, sg):
                """Projections for one sg (2 slots) of `half`."""
                QTs, KTs, V32Q_r = (state[half][k]
                                    for k in ("QTs", "KTs", "V32Q_r"))
                xts = xin.tile([128, 2, 2, 2, B_H], BF16, tag="xts")
                nc.sync.dma_start(out=xts, in_=xH[half, sg])
                wvs = wvin.tile([128, 2, 2, O], BF16, tag="wvs")
                nc.scalar.dma_start(out=wvs, in_=wvH[sg])
                if half == 0:
                    if sg == 0:
                        load_wq_chunk(0)
                    elif sg <= 7:
                        load_wq_chunk(sg)
                # V projections first
                for si in range(2):
                    s = 2 * sg + si
                    psv = mm_ps.tile([128, 2, O], F32, tag="ps")
                    for bc in range(2):
                        for c in range(2):
                            nc.tensor.matmul(
                                psv[:, bc, :],
                                lhsT=xts[:, 1, si, c,
                                         128 * bc:128 * (bc + 1)],
                                rhs=wvs[:, si, c, :],
                                start=(c == 0),
                                stop=(c == 1),
                            )
                    VN = vn.tile([128, 2, O], BF16, tag="VN")
                    evac(VN, psv)
                    # scatter rows {64bc+s, 64bc+32+s} <- VN[:, bc, :]
                    for bc in range(2):
                        nc.gpsimd.dma_start(
                            out=V32Q_r[bc, s::32, :, :],
                            in_=VN[:, bc, :],
                        )
                # Q/K projections; [128, 512] evacuations per (w, slot)
                for w in range(2):
                    for si in range(2):
                        s = 2 * sg + si
                        ps = mm_ps.tile([128, 2, B_H], F32, tag="ps")
                        for t in range(2):
                            for c in range(2):
                                nc.tensor.matmul(
                                    ps[:, t, :],
                                    lhsT=wq[:, s, c, w,
                                            128 * t:128 * (t + 1)],
                                    rhs=xts[:, w, si, c, :],
                                    start=(c == 0),
                                    stop=(c == 1),
                                )
                        dst = QTs if w == 0 else KTs
                        evac(dst[:, :, s, :], ps, 0.25)

            def do_quad(half, gq):
                QTs, KTs, E = (state[half][k] for k in ("QTs", "KTs", "E"))
                lg = mm_ps.tile([128, 4, NK], F32, tag="ps")
                for qi in range(4):
                    g = 4 * gq + qi
                    for c in range(2):
                        for j in range(4):
                            b = 64 * j + g
                            nc.tensor.matmul(
                                lg[32 * j:32 * (j + 1), qi, :],
                                lhsT=QTs[:, c, :, b],
                                rhs=KTs[:, c, :, b],
                                start=(c == 0),
                                stop=(c == 1),
                                tile_position=(0, 32 * j),
                                skip_group_check=True,
                            )
                # softmax without max-subtraction: |logits| <= ~2.5
                nc.scalar.activation(
                    out=E[:, 4 * gq:4 * gq + 4, :].rearrange(
                        "p a b -> p (a b)"),
                    in_=lg.rearrange("p a b -> p (a b)"),
                    func=mybir.ActivationFunctionType.Exp,
                )

            def av_chunk(half, chunk):
                """attn@V + store for groups 8*chunk..8*chunk+8 of `half`."""
                E, rs, V32Q = (state[half][k] for k in ("E", "rs", "V32Q"))
                g0 = 8 * chunk
                sm = smp.tile([128, 8], F32, tag="sm")
                nc.vector.reduce_sum(
                    out=sm, in_=E[:, g0:g0 + 8, :], axis=mybir.AxisListType.X)
                nc.vector.reciprocal(out=rs[:, g0:g0 + 8], in_=sm)
                OUTo = outp.tile([128, 8, O], BF16, tag="OUTo")
                for gg in (0, 4):
                    te4 = etp.tile([128, 4, NK], BF16, tag="te4")
                    nc.vector.transpose(
                        out=te4.rearrange("p a b -> p (a b)"),
                        in_=E[:, g0 + gg:g0 + gg + 4, :].rearrange(
                            "p a b -> p (a b)"),
                    )
                    for g2 in range(4):
                        g = g0 + gg + g2
                        av = mm_ps.tile([128, O], F32, tag="ps")
                        for j in range(4):
                            nc.tensor.matmul(
                                av[32 * j:32 * (j + 1), :],
                                lhsT=te4[32 * j:32 * (j + 1), g2, :],
                                rhs=V32Q[32 * j:32 * (j + 1), g, :],
                                start=True, stop=True,
                                tile_position=(32 * j, 32 * j),
                                skip_group_check=True,
                            )
                        evac(OUTo[:, g - g0, :], av, rs[:, g:g + 1])
                nc.scalar.dma_start(out=out_r[half, :, g0:g0 + 8, :],
                                    in_=OUTo)

            # ---- schedule ----
            open_proj(0)
            for sg in range(16):
                phase_a_sg(0, sg)
            open_soft(0)
            for gq in range(N_QUADS):
                do_quad(0, gq)
            for chunk in range(8):
                av_chunk(0, chunk)
            open_proj(1)
            for sg in range(16):
                phase_a_sg(1, sg)
            open_soft(1)
            for gq in range(N_QUADS):
                do_quad(1, gq)
            for chunk in range(8):
                av_chunk(1, chunk)
    return nc


def _prep_inputs(q, k, query_weight, key_weight, value_weight):
    bf = ml_dtypes.bfloat16
    q = np.asarray(q, dtype=np.float32).astype(bf)
    k = np.asarray(k, dtype=np.float32).astype(bf)

    # xH[ci, half, sg, p, qk, s, c, b] = {q,k}[2sg+s, 512ci+256h+b, 128c+p]
    def pack_x(x):
        t = x.reshape(16, 2, N_CORES, N_HALVES, B_H, 2, 128)
        return t.transpose(2, 3, 0, 6, 1, 5, 4)  # [ci,half,sg,p,s,c,b]
    xAll = np.ascontiguousarray(
        np.stack((pack_x(q), pack_x(k)), axis=4))  # [ci,half,sg,p,qk,s,c,b]
    # wqk[p, slot, c, qk, a]
    ws = np.stack(
        (np.asarray(query_weight, np.float32),
         np.asarray(key_weight, np.float32)), axis=2)  # [n, d, qk, a]
    wqk = np.ascontiguousarray(
        ws.reshape(NQ, 2, 128, 2, A).transpose(2, 0, 1, 3, 4).astype(bf))
    # wvH[sg, p, s, c, o]
    wv = np.ascontiguousarray(
        np.asarray(value_weight, np.float32)
        .reshape(16, 2, 2, 128, O).transpose(0, 3, 1, 2, 4).astype(bf))
    in_maps = []
    for i in range(N_CORES):
        in_maps.append({"xH": np.ascontiguousarray(xAll[i]),
                        "wqk": wqk, "wvH": wv})
    return in_maps


_NC_CACHE = {}


def _get_nc():
    if "nc" not in _NC_CACHE:
        nc = build_kernel()
        nc.finalize()
        _NC_CACHE["nc"] = nc
    return _NC_CACHE["nc"]


def kernel(q, k, query_weight, key_weight, value_weight, _trace=False):
    nc = _get_nc()
    in_maps = _prep_inputs(q, k, query_weight, key_weight, value_weight)
    res = run_bass_kernel_spmd(nc, in_maps, core_ids=list(range(N_CORES)),
                               trace=_trace)
    outs = []
    for i in range(N_CORES):
        o = res.results[i]["out"]  # [half, j, n, g, o]
        outs.append(o.transpose(2, 0, 1, 3, 4).reshape(NQ, BS_CORE, O))
    full = np.concatenate(outs, axis=1).astype(np.float32)
    if _trace:
        return full, res
    return full


# revision 25
# speedup vs baseline: 1.1639x; 1.1639x over previous
"""Slot-attention kernel for Trainium2, SPMD over 8 NeuronCores (v10).

Reference computation (per batch element b):
  query[b,n,:] = q[n,b,:] @ qw[n]          (n = 32 query slots)
  keyp [b,m,:] = k[m,b,:] @ kw[m]          (m = 32 key slots)
  value[b,m,:] = k[m,b,:] @ vw[m]
  logits[b,n,m] = query[b,n,:]·keyp[b,m,:] / 16
  attn = softmax_m(logits)
  out[n,b,:] = sum_m attn[b,n,m] * value[b,m,:]

Sharding: data-parallel over batch (4096 -> 512 per core), weights replicated.

Pipeline layout (per core, two batch halves of 256):
  - dedicated DMA paths: SP HWDGE ring = activations only (so the next
    half's inputs prefetch during phase B); ACT HWDGE ring = weights +
    output stores; gpsimd SWDGE = the V32Q scatter only.
  - q-projection weights stay SBUF-resident (loaded once, chunk-
    interleaved into the first sg iterations); k/v weights stream per sg.
  - phase A: per 2-slot unit, V projections first (psum [b,o]) so the
    V32Q scatter drains early, then Q/K projections (psum [a,b]);
    evacuations are merged [128,1024] ops alternating ScalarE/VectorE;
    the 1/16 temperature is split as 1/4 on Q and 1/4 on K.
  - V32Q[32j+m, g, o] (value for batch 64j+g) is ping-pong buffered
    across halves so half 1's scatter never waits on half 0's attn@V.
  - phase B: logits as col-tiled 4-batch waves + exp on ScalarE (softmax
    without max-subtraction; |logits| <= ~2.5), rowsums 8 groups per DVE
    reduce, attn^T via 4-group 32x32 block transposes, attn@V as diagonal
    tile-packed matmuls, psum->sbuf with folded 1/rowsum, bf16 output via
    one 128-partition DMA per 8 groups.
"""

import numpy as np
import ml_dtypes

import concourse.bass as bass
from concourse import bacc
import concourse.mybir as mybir
import concourse.tile as tile
from concourse.bass_utils import run_bass_kernel_spmd

BF16 = mybir.dt.bfloat16
F32 = mybir.dt.float32

NQ = 32          # query slots
NK = 32          # key slots
D = 256          # input dim (contraction of projections)
A = 256          # attn dim (contraction of logits)
O = 256          # out dim
BS = 4096
N_CORES = 8
BS_CORE = BS // N_CORES   # 512
N_HALVES = 2
B_H = BS_CORE // N_HALVES  # 256
N_GROUPS = B_H // 4        # 64 groups of 4 batches per half
N_QUADS = N_GROUPS // 4    # 16


def build_kernel():
    nc = bacc.Bacc()

    xH = nc.declare_dram_parameter(
        "xH", [N_HALVES, 16, 128, 2, 2, 2, B_H], BF16, isOutput=False
    )  # [half, sg, p, qk, s, c, b]
    wqk = nc.declare_dram_parameter(
        "wqk", [128, NQ, 2, 2, A], BF16, isOutput=False
    )  # [p, slot, c, (q k), a]
    wvH = nc.declare_dram_parameter(
        "wvH", [16, 128, 2, 2, O], BF16, isOutput=False
    )  # [sg, p, s, c, o]
    # batch = 256*half + 64*j + g; host un-permutes to [n, b, o]
    out = nc.declare_dram_parameter(
        "out", [N_HALVES, 4, NQ, N_GROUPS, O], BF16, isOutput=True)
    out_r = out.rearrange("h j n g o -> h (j n) g o")

    with tile.TileContext(nc) as tc:
        with (
            tc.tile_pool(name="wpool", bufs=1) as wpool,
            tc.tile_pool(name="xin", bufs=3) as xin,
            tc.tile_pool(name="wvin", bufs=3) as wvin,
            tc.tile_pool(name="big", bufs=1) as big,
            tc.tile_pool(name="v32", bufs=1) as v32,
            tc.tile_pool(name="vn", bufs=4) as vn,
            tc.tile_pool(name="etp", bufs=4) as etp,
            tc.tile_pool(name="smp", bufs=4) as smp,
            tc.tile_pool(name="outp", bufs=2) as outp,
            tc.tile_pool(name="mm_ps", bufs=3, space="PSUM") as mm_ps,
            tc.tile_pool(name="lg_ps", bufs=2, space="PSUM") as lg_ps,
        ):
            wq = wpool.tile([128, NQ, 2, 2, A], BF16, tag="wq")

            def load_wq_chunk(ch):
                nc.scalar.dma_start(
                    out=wq[:, 4 * ch:4 * (ch + 1)],
                    in_=wqk[:, 4 * ch:4 * (ch + 1)],
                )

            evac_flip = [0]

            def evac(dst, src, scale=None):
                e = evac_flip[0] = 1 - evac_flip[0]
                if scale is None:
                    if e:
                        nc.scalar.copy(out=dst, in_=src)
                    else:
                        nc.vector.tensor_copy(out=dst, in_=src)
                else:
                    if e:
                        nc.scalar.mul(dst, src, scale)
                    else:
                        nc.vector.tensor_scalar_mul(out=dst, in0=src,
                                                    scalar1=scale)

            state = {}

            def open_proj(half):
                QTs = big.tile([128, 2, NQ, B_H], BF16, tag="QTs")
                KTs = big.tile([128, 2, NK, B_H], BF16, tag="KTs")
                V32Q = v32.tile([128, N_GROUPS, O], BF16, tag="V32Q")
                state[half] = {
                    "QTs": QTs, "KTs": KTs, "V32Q": V32Q,
                    "V32Q_r": V32Q.rearrange("(bc q) g o -> bc q g o", bc=2),
                }

            def open_soft(half):
                E = big.tile([128, N_GROUPS, NK], BF16, tag="E")
                rs = big.tile([128, N_GROUPS], F32, tag="rs")
                state[half]["E"] = E
                state[half]["rs"] = rs

            def phase_a_sg(half, sg):
                """Projections for one sg (2 slots) of `half`."""
                QTs, KTs, V32Q_r = (state[half][k]
                                    for k in ("QTs", "KTs", "V32Q_r"))
                xts = xin.tile([128, 2, 2, 2, B_H], BF16, tag="xts")
                nc.sync.dma_start(out=xts, in_=xH[half, sg])
                wvs = wvin.tile([128, 2, 2, O], BF16, tag="wvs")
                nc.scalar.dma_start(out=wvs, in_=wvH[sg])
                if half == 0:
                    if sg == 0:
                        load_wq_chunk(0)
                    elif sg <= 7:
                        load_wq_chunk(sg)
                # V projections first
                psv = mm_ps.tile([128, 2, 2, O], F32, tag="ps")
                for si in range(2):
                    for bc in range(2):
                        for c in range(2):
                            nc.tensor.matmul(
                                psv[:, si, bc, :],
                                lhsT=xts[:, 1, si, c,
                                         128 * bc:128 * (bc + 1)],
                                rhs=wvs[:, si, c, :],
                                start=(c == 0),
                                stop=(c == 1),
                            )
                VN = vn.tile([128, 2, 2, O], BF16, tag="VN")
                evac(VN, psv)
                # scatter rows {64bc+s, 64bc+32+s} <- VN[:, si, bc, :]
                for si in range(2):
                    s = 2 * sg + si
                    for bc in range(2):
                        nc.gpsimd.dma_start(
                            out=V32Q_r[bc, s::32, :, :],
                            in_=VN[:, si, bc, :],
                        )
                # Q/K projections; merged [128, 1024] evacuations
                for w in range(2):
                    ps = mm_ps.tile([128, 2, 2, B_H], F32, tag="ps")
                    for si in range(2):
                        for t in range(2):
                            for c in range(2):
                                nc.tensor.matmul(
                                    ps[:, si, t, :],
                                    lhsT=wq[:, 2 * sg + si, c, w,
                                            128 * t:128 * (t + 1)],
                                    rhs=xts[:, w, si, c, :],
                                    start=(c == 0),
                                    stop=(c == 1),
                                )
                    dst = QTs if w == 0 else KTs
                    evac(dst[:, :, 2 * sg:2 * sg + 2, :]
                         .rearrange("p t s b -> p s t b"), ps, 0.25)

            def do_quad(half, gq):
                QTs, KTs, E = (state[half][k] for k in ("QTs", "KTs", "E"))
                lg = lg_ps.tile([128, 4, NK], F32, tag="lg")
                for qi in range(4):
                    g = 4 * gq + qi
                    for c in range(2):
                        for j in range(4):
                            b = 64 * j + g
                            nc.tensor.matmul(
                                lg[32 * j:32 * (j + 1), qi, :],
                                lhsT=QTs[:, c, :, b],
                                rhs=KTs[:, c, :, b],
                                start=(c == 0),
                                stop=(c == 1),
                                tile_position=(0, 32 * j),
                                skip_group_check=True,
                            )
                # softmax without max-subtraction: |logits| <= ~2.5
                nc.scalar.activation(
                    out=E[:, 4 * gq:4 * gq + 4, :].rearrange(
                        "p a b -> p (a b)"),
                    in_=lg.rearrange("p a b -> p (a b)"),
                    func=mybir.ActivationFunctionType.Exp,
                )

            def av_chunk(half, chunk):
                """attn@V + store for groups 8*chunk..8*chunk+8 of `half`."""
                E, rs, V32Q = (state[half][k] for k in ("E", "rs", "V32Q"))
                g0 = 8 * chunk
                sm = smp.tile([128, 8], F32, tag="sm")
                nc.vector.reduce_sum(
                    out=sm, in_=E[:, g0:g0 + 8, :], axis=mybir.AxisListType.X)
                nc.vector.reciprocal(out=rs[:, g0:g0 + 8], in_=sm)
                OUTo = outp.tile([128, 8, O], BF16, tag="OUTo")
                for gg in (0, 4):
                    te4 = etp.tile([128, 4, NK], BF16, tag="te4")
                    nc.vector.transpose(
                        out=te4.rearrange("p a b -> p (a b)"),
                        in_=E[:, g0 + gg:g0 + gg + 4, :].rearrange(
                            "p a b -> p (a b)"),
                    )
                    for g2 in range(4):
                        g = g0 + gg + g2
                        av = mm_ps.tile([128, O], F32, tag="ps")
                        for j in range(4):
                            nc.tensor.matmul(
                                av[32 * j:32 * (j + 1), :],
                                lhsT=te4[32 * j:32 * (j + 1), g2, :],
                                rhs=V32Q[32 * j:32 * (j + 1), g, :],
                                start=True, stop=True,
                                tile_position=(32 * j, 32 * j),
                                skip_group_check=True,
                            )
                        evac(OUTo[:, g - g0, :], av, rs[:, g:g + 1])
                nc.sync.dma_start(out=out_r[half, :, g0:g0 + 8, :],
                                  in_=OUTo)

            # ---- schedule ----
            open_proj(0)
            for sg in range(16):
                phase_a_sg(0, sg)
            open_soft(0)
            for gq in range(N_QUADS):
                do_quad(0, gq)
            for chunk in range(8):
                av_chunk(0, chunk)
            open_proj(1)
            for sg in range(16):
                phase_a_sg(1, sg)
            open_soft(1)
            for gq in range(N_QUADS):
                do_quad(1, gq)
            for chunk in range(8):
                av_chunk(1, chunk)
    return nc


def _prep_inputs(q, k, query_weight, key_weight, value_weight):
    bf = ml_dtypes.bfloat16
    q = np.asarray(q, dtype=np.float32).astype(bf)
    k = np.asarray(k, dtype=np.float32).astype(bf)

    # xH[ci, half, sg, p, qk, s, c, b] = {q,k}[2sg+s, 512ci+256h+b, 128c+p]
    def pack_x(x):
        t = x.reshape(16, 2, N_CORES, N_HALVES, B_H, 2, 128)
        return t.transpose(2, 3, 0, 6, 1, 5, 4)  # [ci,half,sg,p,s,c,b]
    xAll = np.ascontiguousarray(
        np.stack((pack_x(q), pack_x(k)), axis=4))  # [ci,half,sg,p,qk,s,c,b]
    # wqk[p, slot, c, qk, a]
    ws = np.stack(
        (np.asarray(query_weight, np.float32),
         np.asarray(key_weight, np.float32)), axis=2)  # [n, d, qk, a]
    wqk = np.ascontiguousarray(
        ws.reshape(NQ, 2, 128, 2, A).transpose(2, 0, 1, 3, 4).astype(bf))
    # wvH[sg, p, s, c, o]
    wv = np.ascontiguousarray(
        np.asarray(value_weight, np.float32)
        .reshape(16, 2, 2, 128, O).transpose(0, 3, 1, 2, 4).astype(bf))
    in_maps = []
    for i in range(N_CORES):
        in_maps.append({"xH": np.ascontiguousarray(xAll[i]),
                        "wqk": wqk, "wvH": wv})
    return in_maps


_NC_CACHE = {}


def _get_nc():
    if "nc" not in _NC_CACHE:
        nc = build_kernel()
        nc.finalize()
        _NC_CACHE["nc"] = nc
    return _NC_CACHE["nc"]


def kernel(q, k, query_weight, key_weight, value_weight, _trace=False):
    nc = _get_nc()
    in_maps = _prep_inputs(q, k, query_weight, key_weight, value_weight)
    res = run_bass_kernel_spmd(nc, in_maps, core_ids=list(range(N_CORES)),
                               trace=_trace)
    outs = []
    for i in range(N_CORES):
        o = res.results[i]["out"]  # [half, j, n, g, o]
        outs.append(o.transpose(2, 0, 1, 3, 4).reshape(NQ, BS_CORE, O))
    full = np.concatenate(outs, axis=1).astype(np.float32)
    if _trace:
        return full, res
    return full


# revision 26
# speedup vs baseline: 1.1828x; 1.0163x over previous
"""Slot-attention kernel for Trainium2, SPMD over 8 NeuronCores (v10).

Reference computation (per batch element b):
  query[b,n,:] = q[n,b,:] @ qw[n]          (n = 32 query slots)
  keyp [b,m,:] = k[m,b,:] @ kw[m]          (m = 32 key slots)
  value[b,m,:] = k[m,b,:] @ vw[m]
  logits[b,n,m] = query[b,n,:]·keyp[b,m,:] / 16
  attn = softmax_m(logits)
  out[n,b,:] = sum_m attn[b,n,m] * value[b,m,:]

Sharding: data-parallel over batch (4096 -> 512 per core), weights replicated.

Pipeline layout (per core, two batch halves of 256):
  - dedicated DMA paths: SP HWDGE ring = activations only (so the next
    half's inputs prefetch during phase B); ACT HWDGE ring = weights +
    output stores; gpsimd SWDGE = the V32Q scatter only.
  - q-projection weights stay SBUF-resident (loaded once, chunk-
    interleaved into the first sg iterations); k/v weights stream per sg.
  - phase A: per 2-slot unit, V projections first (psum [b,o]) so the
    V32Q scatter drains early, then Q/K projections (psum [a,b]);
    evacuations are merged [128,1024] ops alternating ScalarE/VectorE;
    the 1/16 temperature is split as 1/4 on Q and 1/4 on K.
  - V32Q[32j+m, g, o] (value for batch 64j+g) is ping-pong buffered
    across halves so half 1's scatter never waits on half 0's attn@V.
  - phase B: logits as col-tiled 4-batch waves + exp on ScalarE (softmax
    without max-subtraction; |logits| <= ~2.5), rowsums 8 groups per DVE
    reduce, attn^T via 4-group 32x32 block transposes, attn@V as diagonal
    tile-packed matmuls, psum->sbuf with folded 1/rowsum, bf16 output via
    one 128-partition DMA per 8 groups.
"""

import numpy as np
import ml_dtypes

import concourse.bass as bass
from concourse import bacc
import concourse.mybir as mybir
import concourse.tile as tile
from concourse.bass_utils import run_bass_kernel_spmd

BF16 = mybir.dt.bfloat16
F32 = mybir.dt.float32

NQ = 32          # query slots
NK = 32          # key slots
D = 256          # input dim (contraction of projections)
A = 256          # attn dim (contraction of logits)
O = 256          # out dim
BS = 4096
N_CORES = 8
BS_CORE = BS // N_CORES   # 512
N_HALVES = 2
B_H = BS_CORE // N_HALVES  # 256
N_GROUPS = B_H // 4        # 64 groups of 4 batches per half
N_QUADS = N_GROUPS // 4    # 16


def build_kernel():
    nc = bacc.Bacc()

    xH = nc.declare_dram_parameter(
        "xH", [N_HALVES, 8, 128, 2, 2, 2, 2, B_H], BF16, isOutput=False
    )  # [half, pair, p, sgi, qk, s, c, b]
    wqk = nc.declare_dram_parameter(
        "wqk", [128, NQ, 2, 2, A], BF16, isOutput=False
    )  # [p, slot, c, (q k), a]
    wvH = nc.declare_dram_parameter(
        "wvH", [8, 128, 2, 2, 2, O], BF16, isOutput=False
    )  # [pair, p, sgi, s, c, o]
    # batch = 256*half + 64*j + g; host un-permutes to [n, b, o]
    out = nc.declare_dram_parameter(
        "out", [N_HALVES, 4, NQ, N_GROUPS, O], BF16, isOutput=True)
    out_r = out.rearrange("h j n g o -> h (j n) g o")

    with tile.TileContext(nc) as tc:
        with (
            tc.tile_pool(name="wpool", bufs=1) as wpool,
            tc.tile_pool(name="xin", bufs=2) as xin,
            tc.tile_pool(name="wvin", bufs=2) as wvin,
            tc.tile_pool(name="big", bufs=1) as big,
            tc.tile_pool(name="v32", bufs=1) as v32,
            tc.tile_pool(name="vn", bufs=4) as vn,
            tc.tile_pool(name="etp", bufs=4) as etp,
            tc.tile_pool(name="smp", bufs=4) as smp,
            tc.tile_pool(name="outp", bufs=2) as outp,
            tc.tile_pool(name="mm_ps", bufs=3, space="PSUM") as mm_ps,
            tc.tile_pool(name="lg_ps", bufs=2, space="PSUM") as lg_ps,
        ):
            wq = wpool.tile([128, NQ, 2, 2, A], BF16, tag="wq")

            def load_wq_chunk(ch):
                nc.scalar.dma_start(
                    out=wq[:, 4 * ch:4 * (ch + 1)],
                    in_=wqk[:, 4 * ch:4 * (ch + 1)],
                )

            evac_flip = [0]

            def evac(dst, src, scale=None):
                e = evac_flip[0] = 1 - evac_flip[0]
                if scale is None:
                    if e:
                        nc.scalar.copy(out=dst, in_=src)
                    else:
                        nc.vector.tensor_copy(out=dst, in_=src)
                else:
                    if e:
                        nc.scalar.mul(dst, src, scale)
                    else:
                        nc.vector.tensor_scalar_mul(out=dst, in0=src,
                                                    scalar1=scale)

            state = {}

            def open_proj(half):
                QTs = big.tile([128, 2, NQ, B_H], BF16, tag="QTs")
                KTs = big.tile([128, 2, NK, B_H], BF16, tag="KTs")
                V32Q = v32.tile([128, N_GROUPS, O], BF16, tag="V32Q")
                state[half] = {
                    "QTs": QTs, "KTs": KTs, "V32Q": V32Q,
                    "V32Q_r": V32Q.rearrange("(bc q) g o -> bc q g o", bc=2),
                }

            def open_soft(half):
                E = big.tile([128, N_GROUPS, NK], BF16, tag="E")
                rs = big.tile([128, N_GROUPS], F32, tag="rs")
                state[half]["E"] = E
                state[half]["rs"] = rs

            def phase_a_pair(half, pr):
                """Projections for one sg-pair (4 slots) of `half`.

                2-sg input transfers -> 8KB/partition descriptors on the
                activation stream (the 4KB ones paced phase A at ~60% of
                the needed input rate)."""
                QTs, KTs, V32Q_r = (state[half][k]
                                    for k in ("QTs", "KTs", "V32Q_r"))
                xts2 = xin.tile([128, 2, 2, 2, 2, B_H], BF16, tag="xts")
                nc.sync.dma_start(out=xts2, in_=xH[half, pr])
                wvs2 = wvin.tile([128, 2, 2, 2, O], BF16, tag="wvs")
                nc.scalar.dma_start(out=wvs2, in_=wvH[pr])
                if half == 0:
                    if pr == 0:
                        load_wq_chunk(0)
                        load_wq_chunk(1)
                    elif pr <= 6:
                        load_wq_chunk(pr + 1)
                for sgi in range(2):
                    sg = 2 * pr + sgi
                    xts = xts2[:, sgi]
                    wvs = wvs2[:, sgi]
                    phase_a_sg_body(half, sg, xts, wvs,
                                    QTs, KTs, V32Q_r)

            def phase_a_sg_body(half, sg, xts, wvs, QTs, KTs, V32Q_r):
                # V projections first
                psv = mm_ps.tile([128, 2, 2, O], F32, tag="ps")
                for si in range(2):
                    for bc in range(2):
                        for c in range(2):
                            nc.tensor.matmul(
                                psv[:, si, bc, :],
                                lhsT=xts[:, 1, si, c,
                                         128 * bc:128 * (bc + 1)],
                                rhs=wvs[:, si, c, :],
                                start=(c == 0),
                                stop=(c == 1),
                            )
                VN = vn.tile([128, 2, 2, O], BF16, tag="VN")
                evac(VN, psv)
                # scatter rows {64bc+s, 64bc+32+s} <- VN[:, si, bc, :]
                for si in range(2):
                    s = 2 * sg + si
                    for bc in range(2):
                        nc.gpsimd.dma_start(
                            out=V32Q_r[bc, s::32, :, :],
                            in_=VN[:, si, bc, :],
                        )
                # Q/K projections; merged [128, 1024] evacuations
                for w in range(2):
                    ps = mm_ps.tile([128, 2, 2, B_H], F32, tag="ps")
                    for si in range(2):
                        for t in range(2):
                            for c in range(2):
                                nc.tensor.matmul(
                                    ps[:, si, t, :],
                                    lhsT=wq[:, 2 * sg + si, c, w,
                                            128 * t:128 * (t + 1)],
                                    rhs=xts[:, w, si, c, :],
                                    start=(c == 0),
                                    stop=(c == 1),
                                )
                    dst = QTs if w == 0 else KTs
                    evac(dst[:, :, 2 * sg:2 * sg + 2, :]
                         .rearrange("p t s b -> p s t b"), ps, 0.25)

            def do_quad(half, gq):
                QTs, KTs, E = (state[half][k] for k in ("QTs", "KTs", "E"))
                lg = lg_ps.tile([128, 4, NK], F32, tag="lg")
                for qi in range(4):
                    g = 4 * gq + qi
                    for c in range(2):
                        for j in range(4):
                            b = 64 * j + g
                            nc.tensor.matmul(
                                lg[32 * j:32 * (j + 1), qi, :],
                                lhsT=QTs[:, c, :, b],
                                rhs=KTs[:, c, :, b],
                                start=(c == 0),
                                stop=(c == 1),
                                tile_position=(0, 32 * j),
                                skip_group_check=True,
                            )
                # softmax without max-subtraction: |logits| <= ~2.5
                nc.scalar.activation(
                    out=E[:, 4 * gq:4 * gq + 4, :].rearrange(
                        "p a b -> p (a b)"),
                    in_=lg.rearrange("p a b -> p (a b)"),
                    func=mybir.ActivationFunctionType.Exp,
                )

            def av_chunk(half, chunk):
                """attn@V + store for groups 8*chunk..8*chunk+8 of `half`."""
                E, rs, V32Q = (state[half][k] for k in ("E", "rs", "V32Q"))
                g0 = 8 * chunk
                sm = smp.tile([128, 8], F32, tag="sm")
                nc.vector.reduce_sum(
                    out=sm, in_=E[:, g0:g0 + 8, :], axis=mybir.AxisListType.X)
                nc.vector.reciprocal(out=rs[:, g0:g0 + 8], in_=sm)
                OUTo = outp.tile([128, 8, O], BF16, tag="OUTo")
                for gg in (0, 4):
                    te4 = etp.tile([128, 4, NK], BF16, tag="te4")
                    nc.vector.transpose(
                        out=te4.rearrange("p a b -> p (a b)"),
                        in_=E[:, g0 + gg:g0 + gg + 4, :].rearrange(
                            "p a b -> p (a b)"),
                    )
                    for g2 in range(4):
                        g = g0 + gg + g2
                        av = mm_ps.tile([128, O], F32, tag="ps")
                        for j in range(4):
                            nc.tensor.matmul(
                                av[32 * j:32 * (j + 1), :],
                                lhsT=te4[32 * j:32 * (j + 1), g2, :],
                                rhs=V32Q[32 * j:32 * (j + 1), g, :],
                                start=True, stop=True,
                                tile_position=(32 * j, 32 * j),
                                skip_group_check=True,
                            )
                        evac(OUTo[:, g - g0, :], av, rs[:, g:g + 1])
                nc.sync.dma_start(out=out_r[half, :, g0:g0 + 8, :],
                                  in_=OUTo)

            # ---- schedule ----
            open_proj(0)
            for pr in range(8):
                phase_a_pair(0, pr)
            open_soft(0)
            for gq in range(N_QUADS):
                do_quad(0, gq)
            for chunk in range(8):
                av_chunk(0, chunk)
            open_proj(1)
            for pr in range(8):
                phase_a_pair(1, pr)
            open_soft(1)
            for gq in range(N_QUADS):
                do_quad(1, gq)
            for chunk in range(8):
                av_chunk(1, chunk)
    return nc


def _prep_inputs(q, k, query_weight, key_weight, value_weight):
    bf = ml_dtypes.bfloat16
    q = np.asarray(q, dtype=np.float32).astype(bf)
    k = np.asarray(k, dtype=np.float32).astype(bf)

    # xH[ci, half, pr, p, sgi, qk, s, c, b]; n = 4pr + 2sgi + s
    def pack_x(x):
        t = x.reshape(8, 2, 2, N_CORES, N_HALVES, B_H, 2, 128)
        # [pr, sgi, s, ci, half, b, c, p] -> [ci, half, pr, p, sgi, s, c, b]
        return t.transpose(3, 4, 0, 7, 1, 2, 6, 5)
    xAll = np.ascontiguousarray(
        np.stack((pack_x(q), pack_x(k)), axis=5))  # qk after sgi
    # wqk[p, slot, c, qk, a]
    ws = np.stack(
        (np.asarray(query_weight, np.float32),
         np.asarray(key_weight, np.float32)), axis=2)  # [n, d, qk, a]
    wqk = np.ascontiguousarray(
        ws.reshape(NQ, 2, 128, 2, A).transpose(2, 0, 1, 3, 4).astype(bf))
    # wvH[pr, p, sgi, s, c, o]
    wv = np.ascontiguousarray(
        np.asarray(value_weight, np.float32)
        .reshape(8, 2, 2, 2, 128, O).transpose(0, 4, 1, 2, 3, 5).astype(bf))
    in_maps = []
    for i in range(N_CORES):
        in_maps.append({"xH": np.ascontiguousarray(xAll[i]),
                        "wqk": wqk, "wvH": wv})
    return in_maps


_NC_CACHE = {}


def _get_nc():
    if "nc" not in _NC_CACHE:
        nc = build_kernel()
        nc.finalize()
        _NC_CACHE["nc"] = nc
    return _NC_CACHE["nc"]


def kernel(q, k, query_weight, key_weight, value_weight, _trace=False):
    nc = _get_nc()
    in_maps = _prep_inputs(q, k, query_weight, key_weight, value_weight)
    res = run_bass_kernel_spmd(nc, in_maps, core_ids=list(range(N_CORES)),
                               trace=_trace)
    outs = []
    for i in range(N_CORES):
        o = res.results[i]["out"]  # [half, j, n, g, o]
        outs.append(o.transpose(2, 0, 1, 3, 4).reshape(NQ, BS_CORE, O))
    full = np.concatenate(outs, axis=1).astype(np.float32)
    if _trace:
        return full, res
    return full
